# revision 20
# baseline (speedup 1.0000x reference)
"""Causal self-attention on 8 Trainium2 NeuronCores (Bass/Tile).

Problem: y = CausalSelfAttention(x; Wq, Wk, Wv, Wo) with
B=4, S=2048, E=1024, H=16 heads of 64, fp32 inputs/outputs.

Sharding (tensor-parallel x data-parallel): core c of 8 handles batch
b = c//2 and head-group g = c%2 (8 of 16 heads). Each core receives
x[b] [S, E], the head-group's columns of Wq/Wk/Wv [E, 512] and rows of
Wo [512, E], and produces a partial output projection [S, E] in bf16.
The host sums the two partials per batch in fp32.

Per-core schedule (one long software pipeline so the PE never idles —
the HAM clock gate re-throttles the PE to 1.2 GHz after ~3.4us of
idle, so PE density is itself a clock-rate optimization):

  prologue: xT DMA-transposes split across both HWDGE rings, weights
    batched on the SWDGE ring; first v / k / q projection chunks.
  for each head pair t: attention(t) — the ACT-engine exp stream is
    the binding resource (~1us per 128-k-tile); the PE's spare slots
    inside attention are filled with the REMAINING projection chunks
    (v tiles, next pair's q/k), emitted interleaved into the qj loops
    so the Tile scheduler's cost model places them in ACT-bound gaps.
  epilogue: output projection accumulated over all 4 pairs in PSUM
    (4 matmuls per [128,512] chunk), bf16 store; the first chunks
    overlap the last pair's attention.

Attention per (t, qj) follows the baseline: two heads packed into
disjoint 64-row PE groups for the ST matmuls (concurrent), one wide
ACTIVATE exp per k-tile amortizing the 352-cycle fixed cost, causal
mask as a triangular multiply on the diagonal subblock, PV with a
ones column so softmax sums ride along in row 64 of the PSUM
accumulator. Sums are normalized via DVE reciprocal on the [1,512]
row + a GpSimd partition-broadcast (the PE outer-product broadcast
and its PSUM bank are gone).
"""

import numpy as np

import concourse.bass as bass
import concourse.mybir as mybir
from concourse import bacc
from concourse.tile import TileContext

FP = mybir.dt.float32
BF = mybir.dt.bfloat16
P = 128


def build(S=2048, E=1024, HPC=8, DH=64, NQ=512):
    GD = HPC * DH  # 512 head dims per core
    KT_E = E // P  # 8 e-tiles
    ST_S = S // P  # 16 s-tiles
    QC = S // NQ  # 4 q-chunks
    DT = GD // P  # 4 head pairs
    QSUB = NQ // P  # 4 k-tiles per q-chunk step
    VW = 65
    SH = S // 2  # xT s-chunk width
    NCH = S // SH  # 2 s-chunks

    assert NQ % P == 0 and S % NQ == 0 and E % P == 0 and GD % P == 0
    assert DH == 64, "head slicing assumes DH=64"

    nc = bacc.Bacc(None, target_bir_lowering=False)
    x_d = nc.dram_tensor("x", [S, E], BF, kind="ExternalInput")
    wq_d = nc.dram_tensor("wq", [E, GD], BF, kind="ExternalInput")
    wk_d = nc.dram_tensor("wk", [E, GD], BF, kind="ExternalInput")
    wv_d = nc.dram_tensor("wv", [E, GD], BF, kind="ExternalInput")
    wo_d = nc.dram_tensor("wo", [GD, E], BF, kind="ExternalInput")
    out_d = nc.dram_tensor("out", [S, E], BF, kind="ExternalOutput")

    with TileContext(nc) as tc:
        with (
            tc.tile_pool(name="consts", bufs=1) as consts,
            tc.tile_pool(name="qkv", bufs=1) as qkv,
            tc.tile_pool(name="xT", bufs=1) as xT_pool,
            tc.tile_pool(name="wbuf", bufs=1) as wbuf,
            tc.tile_pool(name="attT", bufs=1) as attT_pool,
        ):
            ones_bf = consts.tile([P, 8], BF)
            nc.vector.memset(ones_bf[:], 1.0)
            # upper-triangular-inclusive multiplicative mask (valid k <= q)
            ut_mask = consts.tile([P, P], BF)
            nc.gpsimd.memset(ut_mask[:], 0.0)
            nc.gpsimd.affine_select(
                out=ut_mask[:], in_=ut_mask[:],
                compare_op=mybir.AluOpType.is_gt, fill=1.0,
                base=0, pattern=[[-1, P]], channel_multiplier=1,
            )

            qT = [qkv.tile([P, S], BF, tag=f"qT{t}", name=f"qT{t}") for t in range(DT)]
            kT = [qkv.tile([P, S], BF, tag=f"kT{t}", name=f"kT{t}") for t in range(DT)]
            v = [qkv.tile([P, HPC * VW], BF, tag=f"v{t}", name=f"v{t}") for t in range(ST_S)]
            attT = [
                attT_pool.tile([P, S], BF, tag=f"attT{t}", name=f"attT{t}")
                for t in range(DT)
            ]

            xTc = [
                [
                    xT_pool.tile([P, SH], BF, tag=f"xT{t}_{c}", name=f"xT{t}_{c}")
                    for c in range(NCH)
                ]
                for t in range(KT_E)
            ]

            def xT(kt, lo, width):
                c, r = divmod(lo, SH)
                assert r + width <= SH
                return xTc[kt][c][:, r : r + width]

            # ---- input DMA: everything on the sync HWDGE ring in need
            # order (the XBAR transpose unit is shared hardware and mixing
            # rings/modes either races or gets serialized behind the whole
            # transpose stream — one explicitly-ordered ring is both safe
            # and schedulable) ------------------------------------------
            hwdge = [nc.sync, nc.scalar]

            def w3_load(name, ap3):
                wt = wbuf.tile(
                    [P, ap3.shape[1] * ap3.shape[2]], BF, tag=name, name=name
                )
                w3 = wt.rearrange("p (k c) -> p k c", c=ap3.shape[2])
                nc.sync.dma_start(out=w3[:, :, :], in_=ap3)
                return w3

            # single ring, need-order: Tile's 8 DMA-completion sem lanes
            # are handed out round-robin in order, and each DMA waits on
            # its lane's previous user — so cross-ring interleaves
            # serialize anyway, and one explicitly-ordered ring wins
            for et in range(KT_E):
                nc.sync.dma_start(
                    out=xTc[et][0][:],
                    in_=x_d[0:SH, et * P : (et + 1) * P],
                    transpose=True,
                )
            wv3 = w3_load("wv", wv_d.rearrange("(k p) c -> p k c", p=P))
            wqk = {}
            for pname, w_d in (("k", wk_d), ("q", wq_d)):
                wqk[pname, 0] = w3_load(
                    f"w{pname}0",
                    w_d[:, 0:P].rearrange("(k p) c -> p k c", p=P),
                )
            for t in range(1, DT):
                for pname, w_d in (("k", wk_d), ("q", wq_d)):
                    wqk[pname, t] = w3_load(
                        f"w{pname}{t}",
                        w_d[:, t * P : (t + 1) * P].rearrange(
                            "(k p) c -> p k c", p=P
                        ),
                    )
            wo_sb = [
                wbuf.tile([P, E], BF, tag=f"wo{t}", name=f"wo{t}")
                for t in range(DT)
            ]
            for t in range(DT):
                nc.sync.dma_start(
                    out=wo_sb[t][:], in_=wo_d[t * P : (t + 1) * P, :]
                )
            for et in range(KT_E):
                nc.sync.dma_start(
                    out=xTc[et][1][:],
                    in_=x_d[SH : 2 * SH, et * P : (et + 1) * P],
                    transpose=True,
                )

            # ---- projection chunk emitters --------------------------------
            def v_chunk(pool, st, copy_eng):
                ps = pool.tile([P, GD], FP, tag="pj")
                for kt in range(KT_E):
                    nc.tensor.matmul(
                        ps[:],
                        lhsT=xT(kt, st * P, P),
                        rhs=wv3[:, kt, :],
                        start=(kt == 0),
                        stop=(kt == KT_E - 1),
                    )
                vst3 = v[st].rearrange("p (h c) -> p h c", c=VW)
                nc.vector.tensor_copy(
                    vst3[:, 0:HPC, 64:65], ones_bf[:, 0:HPC, None]
                )
                copy_eng(
                    vst3[:, :, 0:64], ps[:].rearrange("p (h c) -> p h c", c=DH)
                )

            def qk_chunk(pool, pname, t, nsc, copy_eng):
                dstT = kT if pname == "k" else qT
                ps = pool.tile([P, NQ], FP, tag="pj")
                w3 = wqk[pname, t]
                for kt in range(KT_E):
                    nc.tensor.matmul(
                        ps[:],
                        lhsT=w3[:, kt, :],
                        rhs=xT(kt, nsc * NQ, NQ),
                        start=(kt == 0),
                        stop=(kt == KT_E - 1),
                    )
                copy_eng(dstT[t][:, nsc * NQ : (nsc + 1) * NQ], ps[:])

            def scalar_copy(dst, src):
                nc.scalar.copy(dst, src)

            def vector_copy(dst, src):
                nc.vector.tensor_copy(dst, src)

            # ---- prologue: just enough projections to start attention ----
            # qk copies ride the idle ACT engine; the 3D-strided v copies
            # stay on the DVE
            with tc.tile_pool(name="pj_psum", bufs=4, space="PSUM") as pj_psum:
                qk_chunk(pj_psum, "k", 0, 0, scalar_copy)
                qk_chunk(pj_psum, "q", 0, 0, scalar_copy)
                for st in range(4):
                    v_chunk(pj_psum, st, vector_copy)

            # filler schedule: remaining projection chunks, emitted into
            # the attention qj loops as PE gap-fillers.  att(0) finishes
            # v and pair-0's q/k tails + pair-1's q/k; att(1) -> pair 2;
            # att(2) -> pair 3; att(3) overlaps the output projection.
            fillers = {
                (0, 0): [("v", 4), ("v", 5), ("v", 6), ("v", 7),
                         ("k", 0, 1), ("q", 0, 1)],
                (0, 1): [("v", 8), ("v", 9), ("v", 10), ("v", 11),
                         ("k", 0, 2), ("q", 0, 2)],
                (0, 2): [("v", 12), ("v", 13), ("v", 14), ("v", 15),
                         ("k", 0, 3), ("q", 0, 3)],
                (0, 3): [("k", 1, 0), ("q", 1, 0), ("k", 1, 1), ("q", 1, 1)],
                (1, 0): [("k", 1, 2), ("q", 1, 2), ("k", 1, 3), ("q", 1, 3)],
                (1, 1): [("k", 2, 0), ("q", 2, 0)],
                (1, 2): [("k", 2, 1), ("q", 2, 1), ("k", 2, 2), ("q", 2, 2)],
                (1, 3): [("k", 2, 3), ("q", 2, 3)],
                (2, 0): [("k", 3, 0), ("q", 3, 0)],
                (2, 1): [("k", 3, 1), ("q", 3, 1)],
                (2, 2): [("k", 3, 2), ("q", 3, 2)],
                (2, 3): [("k", 3, 3), ("q", 3, 3)],
            }

            # ---- attention + epilogue (PSUM: st 4 banks, av 2, sc 2) ----
            with (
                tc.tile_pool(name="st_psum", bufs=2, space="PSUM") as st_psum,
                tc.tile_pool(name="av_psum", bufs=2, space="PSUM") as av_psum,
                tc.tile_pool(name="sc_psum", bufs=2, space="PSUM") as sc_psum,
                tc.tile_pool(name="pt_sb", bufs=10) as pt_pool,
                tc.tile_pool(name="nrm_sb", bufs=4) as nrm_pool,
                tc.tile_pool(name="ostage", bufs=4) as ostage,
            ):
                di = [0]

                def outproj_chunk(st, nj):
                    po = sc_psum.tile([P, NQ], FP, tag="pj")
                    for tt in range(DT):
                        nc.tensor.matmul(
                            po[:],
                            lhsT=attT[tt][:, st * P : (st + 1) * P],
                            rhs=wo_sb[tt][:, nj * NQ : (nj + 1) * NQ],
                            start=(tt == 0), stop=(tt == DT - 1),
                        )
                    og = ostage.tile([P, NQ], BF, tag="og")
                    nc.vector.tensor_copy(og[:], po[:])
                    hwdge[di[0] % 2].dma_start(
                        out=out_d[st * P : (st + 1) * P,
                                  nj * NQ : (nj + 1) * NQ],
                        in_=og[:],
                    )
                    di[0] += 1

                def emit_fillers(t, qj):
                    for f in fillers.get((t, qj), ()):
                        if f[0] == "v":
                            v_chunk(sc_psum, f[1], vector_copy)
                        else:
                            qk_chunk(sc_psum, f[0], f[1], f[2], vector_copy)


                for t in range(DT):
                    # head pair (2t, 2t+1): partition rows 0-63 / 64-127 of
                    # kT[t]/qT[t].  The two ST matmuls use disjoint 64-row
                    # groups of the PE array and run concurrently.
                    attT_t = attT[t]
                    for qj in range(QC):
                        avs = [
                            av_psum.tile([VW, NQ], FP, tag="av",
                                         name=f"av{t}_{qj}_{half}")
                            for half in range(2)
                        ]
                        n_tiles = QSUB * qj + QSUB
                        kmax = n_tiles - 1
                        for ki in range(n_tiles):
                            d = ki - QSUB * qj
                            off = P * d if d > 0 else 0
                            stp = st_psum.tile([P, 2 * NQ], FP, tag="st")
                            for half in range(2):
                                pr = 64 * half
                                nc.tensor.matmul(
                                    stp[:, half * NQ + off : (half + 1) * NQ],
                                    lhsT=kT[t][pr : pr + 64, ki * P : (ki + 1) * P],
                                    rhs=qT[t][pr : pr + 64, qj * NQ + off : (qj + 1) * NQ],
                                    start=True, stop=True,
                                )
                            pt = pt_pool.tile([P, 2 * NQ], BF, tag="pt")
                            if off == 0:
                                nc.scalar.activation(
                                    pt[:, 0 : 2 * NQ], stp[:, 0 : 2 * NQ],
                                    mybir.ActivationFunctionType.Exp,
                                    scale=0.125,
                                )
                            else:
                                # one ACTIVATE over both heads' valid spans
                                # via a strided AP; dead cols are never read
                                # (the PV matmuls below are col-restricted)
                                pt2 = pt.rearrange("p (k c) -> p k c", c=NQ)
                                st2 = stp.rearrange("p (k c) -> p k c", c=NQ)
                                nc.scalar.activation(
                                    pt2[:, :, off:NQ], st2[:, :, off:NQ],
                                    mybir.ActivationFunctionType.Exp,
                                    scale=0.125,
                                )
                            if d >= 0:
                                # mask on GpSimd: keeps the DVE queue clear
                                # so the norm chain (which releases the av
                                # bank the next qj's PV blocks on) runs
                                # as soon as its inputs land
                                for half in range(2):
                                    nc.gpsimd.tensor_tensor(
                                        pt[:, half * NQ + off : half * NQ + off + P],
                                        pt[:, half * NQ + off : half * NQ + off + P],
                                        ut_mask[:],
                                        mybir.AluOpType.mult,
                                    )
                            for half in range(2):
                                h = 2 * t + half
                                # col-restricted: cols < off are fully masked
                                # and were already initialized by ki=0
                                nc.tensor.matmul(
                                    avs[half][:, off:NQ],
                                    lhsT=v[ki][:, VW * h : VW * h + VW],
                                    rhs=pt[:, half * NQ + off : (half + 1) * NQ],
                                    start=(ki == 0), stop=(ki == kmax),
                                    skip_group_check=True,
                                )
                        for half in range(2):
                            pr = 64 * half
                            av = avs[half]
                            # softmax sums ride in row 64; move them to
                            # partition 0 (plain copy handles the cross-
                            # partition hop; custom DVE ops are lane-wired),
                            # reciprocal there, broadcast on the idle GpSimd
                            sums = nrm_pool.tile([1, NQ], FP, tag="sums")
                            nc.vector.tensor_copy(sums[:], av[64:65, :])
                            sums_r = nrm_pool.tile([1, NQ], FP, tag="sumr")
                            nc.vector.reciprocal_approx_fast(
                                sums_r[:], sums[:]
                            )
                            bc = nrm_pool.tile([64, NQ], FP, tag="bc")
                            nc.gpsimd.partition_broadcast(
                                bc[:], sums_r[:], channels=64
                            )
                            nc.vector.tensor_tensor(
                                attT_t[pr : pr + 64, qj * NQ : (qj + 1) * NQ],
                                av[0:64, :],
                                bc[:],
                                mybir.AluOpType.mult,
                            )
                        emit_fillers(t, qj)

                # output projection after the attention loops: the chunks'
                # attT-slice deps resolve qj-by-qj, so the scheduler pulls
                # them into att(3)'s ACT-bound slack on its own
                for st in range(ST_S):
                    for nj in range(E // NQ):
                        outproj_chunk(st, nj)


    nc.compile()
    return nc


_NC_CACHE = {}


def _get_nc():
    if "nc" not in _NC_CACHE:
        _NC_CACHE["nc"] = build()
    return _NC_CACHE["nc"]


B, S, E, H, DH = 4, 2048, 1024, 16, 64
GD = (H // 2) * DH  # 512 per-core head dims


def _in_maps(x, Wq, Wk, Wv, Wo):
    import ml_dtypes

    bf = ml_dtypes.bfloat16
    maps = []
    for c in range(8):
        b, g = c // 2, c % 2
        sl = slice(g * GD, (g + 1) * GD)
        maps.append({
            "x": x[b].astype(bf),
            "wq": Wq[:, sl].astype(bf),
            "wk": Wk[:, sl].astype(bf),
            "wv": Wv[:, sl].astype(bf),
            "wo": Wo[sl, :].astype(bf),
        })
    return maps


def kernel(x, Wq, Wk, Wv, Wo):
    from concourse.bass_utils import run_bass_kernel_spmd

    x = np.asarray(x, dtype=np.float32)
    Wq = np.asarray(Wq, dtype=np.float32)
    Wk = np.asarray(Wk, dtype=np.float32)
    Wv = np.asarray(Wv, dtype=np.float32)
    Wo = np.asarray(Wo, dtype=np.float32)

    res = run_bass_kernel_spmd(
        _get_nc(), _in_maps(x, Wq, Wk, Wv, Wo), list(range(8))
    )

    out = np.empty((B, S, E), np.float32)
    for b in range(B):
        out[b] = res.results[2 * b]["out"].astype(np.float32) + res.results[
            2 * b + 1
        ]["out"].astype(np.float32)
    return out


# revision 21
# speedup vs baseline: 1.7070x; 1.7070x over previous
"""Causal self-attention on 8 Trainium2 NeuronCores (Bass/Tile).

Problem: y = CausalSelfAttention(x; Wq, Wk, Wv, Wo) with
B=4, S=2048, E=1024, H=16 heads of 64, fp32 inputs/outputs.

Sharding (tensor-parallel x data-parallel): core c of 8 handles batch
b = c//2 and head-group g = c%2 (8 of 16 heads). Each core receives
x[b] [S, E], the head-group's columns of Wq/Wk/Wv [E, 512] and rows of
Wo [512, E], and produces a partial output projection [S, E] in bf16.
The host sums the two partials per batch in fp32.

Per-core schedule (one long software pipeline so the PE never idles —
the HAM clock gate re-throttles the PE to 1.2 GHz after ~3.4us of
idle, so PE density is itself a clock-rate optimization):

  prologue: xT DMA-transposes split across both HWDGE rings, weights
    batched on the SWDGE ring; first v / k / q projection chunks.
  for each head pair t: attention(t) — the ACT-engine exp stream is
    the binding resource (~1us per 128-k-tile); the PE's spare slots
    inside attention are filled with the REMAINING projection chunks
    (v tiles, next pair's q/k), emitted interleaved into the qj loops
    so the Tile scheduler's cost model places them in ACT-bound gaps.
  epilogue: output projection accumulated over all 4 pairs in PSUM
    (4 matmuls per [128,512] chunk), bf16 store; the first chunks
    overlap the last pair's attention.

Attention per (t, qj) follows the baseline: two heads packed into
disjoint 64-row PE groups for the ST matmuls (concurrent), one wide
ACTIVATE exp per k-tile amortizing the 352-cycle fixed cost, causal
mask as a triangular multiply on the diagonal subblock, PV with a
ones column so softmax sums ride along in row 64 of the PSUM
accumulator. Sums are normalized via DVE reciprocal on the [1,512]
row + a GpSimd partition-broadcast (the PE outer-product broadcast
and its PSUM bank are gone).
"""

import numpy as np

import concourse.bass as bass
import concourse.mybir as mybir
from concourse import bacc
from concourse.tile import TileContext

FP = mybir.dt.float32
BF = mybir.dt.bfloat16
P = 128


def build(S=2048, E=1024, HPC=8, DH=64, NQ=512):
    GD = HPC * DH  # 512 head dims per core
    KT_E = E // P  # 8 e-tiles
    ST_S = S // P  # 16 s-tiles
    QC = S // NQ  # 4 q-chunks
    DT = GD // P  # 4 head pairs
    QSUB = NQ // P  # 4 k-tiles per q-chunk step
    VW = 65
    SH = S // 2  # xT s-chunk width
    NCH = S // SH  # 2 s-chunks

    assert NQ % P == 0 and S % NQ == 0 and E % P == 0 and GD % P == 0
    assert DH == 64, "head slicing assumes DH=64"

    nc = bacc.Bacc(None, target_bir_lowering=False)
    x_d = nc.dram_tensor("x", [S, E], BF, kind="ExternalInput")
    wq_d = nc.dram_tensor("wq", [E, GD], BF, kind="ExternalInput")
    wk_d = nc.dram_tensor("wk", [E, GD], BF, kind="ExternalInput")
    wv_d = nc.dram_tensor("wv", [E, GD], BF, kind="ExternalInput")
    wo_d = nc.dram_tensor("wo", [GD, E], BF, kind="ExternalInput")
    out_d = nc.dram_tensor("out", [S, E], BF, kind="ExternalOutput")

    with TileContext(nc) as tc:
        with (
            tc.tile_pool(name="consts", bufs=1) as consts,
            tc.tile_pool(name="qkv", bufs=1) as qkv,
            tc.tile_pool(name="xT", bufs=1) as xT_pool,
            tc.tile_pool(name="wbuf", bufs=1) as wbuf,
            tc.tile_pool(name="attT", bufs=1) as attT_pool,
        ):
            ones_bf = consts.tile([P, 8], BF)
            nc.vector.memset(ones_bf[:], 1.0)
            # upper-triangular-inclusive multiplicative mask (valid k <= q)
            ut_mask = consts.tile([P, P], BF)
            nc.gpsimd.memset(ut_mask[:], 0.0)
            nc.gpsimd.affine_select(
                out=ut_mask[:], in_=ut_mask[:],
                compare_op=mybir.AluOpType.is_gt, fill=1.0,
                base=0, pattern=[[-1, P]], channel_multiplier=1,
            )

            qT = [qkv.tile([P, S], BF, tag=f"qT{t}", name=f"qT{t}") for t in range(DT)]
            kT = [qkv.tile([P, S], BF, tag=f"kT{t}", name=f"kT{t}") for t in range(DT)]
            v = [qkv.tile([P, HPC * VW], BF, tag=f"v{t}", name=f"v{t}") for t in range(ST_S)]
            attT = [
                attT_pool.tile([P, S], BF, tag=f"attT{t}", name=f"attT{t}")
                for t in range(DT)
            ]

            xTc = [
                [
                    xT_pool.tile([P, SH], BF, tag=f"xT{t}_{c}", name=f"xT{t}_{c}")
                    for c in range(NCH)
                ]
                for t in range(KT_E)
            ]

            def xT(kt, lo, width):
                c, r = divmod(lo, SH)
                assert r + width <= SH
                return xTc[kt][c][:, r : r + width]

            # ---- input DMA: everything on the sync HWDGE ring in need
            # order (the XBAR transpose unit is shared hardware and mixing
            # rings/modes either races or gets serialized behind the whole
            # transpose stream — one explicitly-ordered ring is both safe
            # and schedulable) ------------------------------------------
            hwdge = [nc.sync, nc.scalar]

            def w3_load(name, ap3):
                wt = wbuf.tile(
                    [P, ap3.shape[1] * ap3.shape[2]], BF, tag=name, name=name
                )
                w3 = wt.rearrange("p (k c) -> p k c", c=ap3.shape[2])
                nc.sync.dma_start(out=w3[:, :, :], in_=ap3)
                return w3

            # single ring, need-order: Tile's 8 DMA-completion sem lanes
            # are handed out round-robin in order, and each DMA waits on
            # its lane's previous user — so cross-ring interleaves
            # serialize anyway, and one explicitly-ordered ring wins
            for et in range(KT_E):
                nc.sync.dma_start(
                    out=xTc[et][0][:],
                    in_=x_d[0:SH, et * P : (et + 1) * P],
                    transpose=True,
                )
            wv3 = w3_load("wv", wv_d.rearrange("(k p) c -> p k c", p=P))
            wqk = {}
            for pname, w_d in (("k", wk_d), ("q", wq_d)):
                wqk[pname, 0] = w3_load(
                    f"w{pname}0",
                    w_d[:, 0:P].rearrange("(k p) c -> p k c", p=P),
                )
            for t in range(1, DT):
                for pname, w_d in (("k", wk_d), ("q", wq_d)):
                    wqk[pname, t] = w3_load(
                        f"w{pname}{t}",
                        w_d[:, t * P : (t + 1) * P].rearrange(
                            "(k p) c -> p k c", p=P
                        ),
                    )
            wo_sb = [
                wbuf.tile([P, E], BF, tag=f"wo{t}", name=f"wo{t}")
                for t in range(DT)
            ]
            for t in range(DT):
                nc.sync.dma_start(
                    out=wo_sb[t][:], in_=wo_d[t * P : (t + 1) * P, :]
                )
            for et in range(KT_E):
                nc.sync.dma_start(
                    out=xTc[et][1][:],
                    in_=x_d[SH : 2 * SH, et * P : (et + 1) * P],
                    transpose=True,
                )

            # ---- projection chunk emitters --------------------------------
            def v_chunk(pool, st, copy_eng):
                ps = pool.tile([P, GD], FP, tag="pj")
                for kt in range(KT_E):
                    nc.tensor.matmul(
                        ps[:],
                        lhsT=xT(kt, st * P, P),
                        rhs=wv3[:, kt, :],
                        start=(kt == 0),
                        stop=(kt == KT_E - 1),
                    )
                vst3 = v[st].rearrange("p (h c) -> p h c", c=VW)
                nc.vector.tensor_copy(
                    vst3[:, 0:HPC, 64:65], ones_bf[:, 0:HPC, None]
                )
                copy_eng(
                    vst3[:, :, 0:64], ps[:].rearrange("p (h c) -> p h c", c=DH)
                )

            def qk_chunk(pool, pname, t, nsc, copy_eng):
                dstT = kT if pname == "k" else qT
                ps = pool.tile([P, NQ], FP, tag="pj")
                w3 = wqk[pname, t]
                for kt in range(KT_E):
                    nc.tensor.matmul(
                        ps[:],
                        lhsT=w3[:, kt, :],
                        rhs=xT(kt, nsc * NQ, NQ),
                        start=(kt == 0),
                        stop=(kt == KT_E - 1),
                    )
                copy_eng(dstT[t][:, nsc * NQ : (nsc + 1) * NQ], ps[:])

            def scalar_copy(dst, src):
                nc.scalar.copy(dst, src)

            def vector_copy(dst, src):
                nc.vector.tensor_copy(dst, src)

            # ---- prologue: just enough projections to start attention ----
            # qk copies ride the idle ACT engine; the 3D-strided v copies
            # stay on the DVE
            with tc.tile_pool(name="pj_psum", bufs=4, space="PSUM") as pj_psum:
                qk_chunk(pj_psum, "k", 0, 0, scalar_copy)
                qk_chunk(pj_psum, "q", 0, 0, scalar_copy)
                for st in range(4):
                    v_chunk(pj_psum, st, vector_copy)

            # filler schedule: remaining projection chunks, emitted into
            # the attention qj loops as PE gap-fillers.  att(0) finishes
            # v and pair-0's q/k tails + pair-1's q/k; att(1) -> pair 2;
            # att(2) -> pair 3; att(3) overlaps the output projection.
            fillers = {
                (0, 0): [("v", 4), ("v", 5), ("v", 6), ("v", 7),
                         ("k", 0, 1), ("q", 0, 1)],
                (0, 1): [("v", 8), ("v", 9), ("v", 10), ("v", 11),
                         ("k", 0, 2), ("q", 0, 2)],
                (0, 2): [("v", 12), ("v", 13), ("v", 14), ("v", 15),
                         ("k", 0, 3), ("q", 0, 3)],
                (0, 3): [("k", 1, 0), ("q", 1, 0), ("k", 1, 1), ("q", 1, 1)],
                (1, 0): [("k", 1, 2), ("q", 1, 2), ("k", 1, 3), ("q", 1, 3)],
                (1, 1): [("k", 2, 0), ("q", 2, 0)],
                (1, 2): [("k", 2, 1), ("q", 2, 1), ("k", 2, 2), ("q", 2, 2)],
                (1, 3): [("k", 2, 3), ("q", 2, 3)],
                (2, 0): [("k", 3, 0), ("q", 3, 0)],
                (2, 1): [("k", 3, 1), ("q", 3, 1)],
                (2, 2): [("k", 3, 2), ("q", 3, 2)],
                (2, 3): [("k", 3, 3), ("q", 3, 3)],
            }

            # ---- attention + epilogue (PSUM: st 4 banks, av 2, sc 2) ----
            with (
                tc.tile_pool(name="st_psum", bufs=2, space="PSUM") as st_psum,
                tc.tile_pool(name="av_psum", bufs=2, space="PSUM") as av_psum,
                tc.tile_pool(name="sc_psum", bufs=2, space="PSUM") as sc_psum,
                tc.tile_pool(name="pt_sb", bufs=10) as pt_pool,
                tc.tile_pool(name="nrm_sb", bufs=4) as nrm_pool,
                tc.tile_pool(name="ostage", bufs=4) as ostage,
            ):
                di = [0]

                def outproj_chunk(st, nj):
                    po = sc_psum.tile([P, NQ], FP, tag="pj")
                    for tt in range(DT):
                        nc.tensor.matmul(
                            po[:],
                            lhsT=attT[tt][:, st * P : (st + 1) * P],
                            rhs=wo_sb[tt][:, nj * NQ : (nj + 1) * NQ],
                            start=(tt == 0), stop=(tt == DT - 1),
                        )
                    og = ostage.tile([P, NQ], BF, tag="og")
                    nc.vector.tensor_copy(og[:], po[:])
                    hwdge[di[0] % 2].dma_start(
                        out=out_d[st * P : (st + 1) * P,
                                  nj * NQ : (nj + 1) * NQ],
                        in_=og[:],
                    )
                    di[0] += 1

                def emit_fillers(t, qj):
                    for f in fillers.get((t, qj), ()):
                        if f[0] == "v":
                            v_chunk(sc_psum, f[1], vector_copy)
                        else:
                            qk_chunk(sc_psum, f[0], f[1], f[2], vector_copy)


                for t in range(DT):
                    # head pair (2t, 2t+1): partition rows 0-63 / 64-127 of
                    # kT[t]/qT[t].  The two ST matmuls use disjoint 64-row
                    # groups of the PE array and run concurrently.
                    attT_t = attT[t]
                    for qj in range(QC):
                        avs = [
                            av_psum.tile([VW, NQ], FP, tag="av",
                                         name=f"av{t}_{qj}_{half}")
                            for half in range(2)
                        ]
                        n_tiles = QSUB * qj + QSUB
                        kmax = n_tiles - 1
                        for ki in range(n_tiles):
                            d = ki - QSUB * qj
                            off = P * d if d > 0 else 0
                            stp = st_psum.tile([P, 2 * NQ], FP, tag="st")
                            for half in range(2):
                                pr = 64 * half
                                nc.tensor.matmul(
                                    stp[:, half * NQ + off : (half + 1) * NQ],
                                    lhsT=kT[t][pr : pr + 64, ki * P : (ki + 1) * P],
                                    rhs=qT[t][pr : pr + 64, qj * NQ + off : (qj + 1) * NQ],
                                    start=True, stop=True,
                                )
                            pt = pt_pool.tile([P, 2 * NQ], BF, tag="pt")
                            if off == 0:
                                nc.scalar.activation(
                                    pt[:, 0 : 2 * NQ], stp[:, 0 : 2 * NQ],
                                    mybir.ActivationFunctionType.Exp,
                                    scale=0.125,
                                )
                            else:
                                # one ACTIVATE over both heads' valid spans
                                # via a strided AP; dead cols are never read
                                # (the PV matmuls below are col-restricted)
                                pt2 = pt.rearrange("p (k c) -> p k c", c=NQ)
                                st2 = stp.rearrange("p (k c) -> p k c", c=NQ)
                                nc.scalar.activation(
                                    pt2[:, :, off:NQ], st2[:, :, off:NQ],
                                    mybir.ActivationFunctionType.Exp,
                                    scale=0.125,
                                )
                            if d >= 0:
                                for half in range(2):
                                    nc.vector.tensor_tensor(
                                        pt[:, half * NQ + off : half * NQ + off + P],
                                        pt[:, half * NQ + off : half * NQ + off + P],
                                        ut_mask[:],
                                        mybir.AluOpType.mult,
                                    )
                            for half in range(2):
                                h = 2 * t + half
                                # col-restricted: cols < off are fully masked
                                # and were already initialized by ki=0
                                nc.tensor.matmul(
                                    avs[half][:, off:NQ],
                                    lhsT=v[ki][:, VW * h : VW * h + VW],
                                    rhs=pt[:, half * NQ + off : (half + 1) * NQ],
                                    start=(ki == 0), stop=(ki == kmax),
                                    skip_group_check=True,
                                )
                        for half in range(2):
                            pr = 64 * half
                            av = avs[half]
                            # softmax sums ride in row 64; move them to
                            # partition 0 (plain copy handles the cross-
                            # partition hop; custom DVE ops are lane-wired),
                            # reciprocal there, broadcast on the idle GpSimd
                            sums = nrm_pool.tile([1, NQ], FP, tag="sums")
                            nc.vector.tensor_copy(sums[:], av[64:65, :])
                            sums_r = nrm_pool.tile([1, NQ], FP, tag="sumr")
                            nc.vector.reciprocal_approx_fast(
                                sums_r[:], sums[:]
                            )
                            bc = nrm_pool.tile([64, NQ], FP, tag="bc")
                            nc.gpsimd.partition_broadcast(
                                bc[:], sums_r[:], channels=64
                            )
                            nc.vector.tensor_tensor(
                                attT_t[pr : pr + 64, qj * NQ : (qj + 1) * NQ],
                                av[0:64, :],
                                bc[:],
                                mybir.AluOpType.mult,
                            )
                        emit_fillers(t, qj)

                # output projection after the attention loops: the chunks'
                # attT-slice deps resolve qj-by-qj, so the scheduler pulls
                # them into att(3)'s ACT-bound slack on its own
                for st in range(ST_S):
                    for nj in range(E // NQ):
                        outproj_chunk(st, nj)


    nc.compile()
    return nc


_NC_CACHE = {}


def _get_nc():
    if "nc" not in _NC_CACHE:
        _NC_CACHE["nc"] = build()
    return _NC_CACHE["nc"]


B, S, E, H, DH = 4, 2048, 1024, 16, 64
GD = (H // 2) * DH  # 512 per-core head dims


def _in_maps(x, Wq, Wk, Wv, Wo):
    import ml_dtypes

    bf = ml_dtypes.bfloat16
    maps = []
    for c in range(8):
        b, g = c // 2, c % 2
        sl = slice(g * GD, (g + 1) * GD)
        maps.append({
            "x": x[b].astype(bf),
            "wq": Wq[:, sl].astype(bf),
            "wk": Wk[:, sl].astype(bf),
            "wv": Wv[:, sl].astype(bf),
            "wo": Wo[sl, :].astype(bf),
        })
    return maps


def kernel(x, Wq, Wk, Wv, Wo):
    from concourse.bass_utils import run_bass_kernel_spmd

    x = np.asarray(x, dtype=np.float32)
    Wq = np.asarray(Wq, dtype=np.float32)
    Wk = np.asarray(Wk, dtype=np.float32)
    Wv = np.asarray(Wv, dtype=np.float32)
    Wo = np.asarray(Wo, dtype=np.float32)

    res = run_bass_kernel_spmd(
        _get_nc(), _in_maps(x, Wq, Wk, Wv, Wo), list(range(8))
    )

    out = np.empty((B, S, E), np.float32)
    for b in range(B):
        out[b] = res.results[2 * b]["out"].astype(np.float32) + res.results[
            2 * b + 1
        ]["out"].astype(np.float32)
    return out


# revision 25
# speedup vs baseline: 1.7370x; 1.0176x over previous
"""Causal self-attention on 8 Trainium2 NeuronCores (Bass/Tile).

Problem: y = CausalSelfAttention(x; Wq, Wk, Wv, Wo) with
B=4, S=2048, E=1024, H=16 heads of 64, fp32 inputs/outputs.

Sharding (tensor-parallel x data-parallel): core c of 8 handles batch
b = c//2 and head-group g = c%2 (8 of 16 heads). Each core receives
x[b] [S, E], the head-group's columns of Wq/Wk/Wv [E, 512] and rows of
Wo [512, E], and produces a partial output projection [S, E] in bf16.
The host sums the two partials per batch in fp32.

Per-core schedule (one long software pipeline so the PE never idles —
the HAM clock gate re-throttles the PE to 1.2 GHz after ~3.4us of
idle, so PE density is itself a clock-rate optimization):

  prologue: xT DMA-transposes split across both HWDGE rings, weights
    batched on the SWDGE ring; first v / k / q projection chunks.
  for each head pair t: attention(t) — the ACT-engine exp stream is
    the binding resource (~1us per 128-k-tile); the PE's spare slots
    inside attention are filled with the REMAINING projection chunks
    (v tiles, next pair's q/k), emitted interleaved into the qj loops
    so the Tile scheduler's cost model places them in ACT-bound gaps.
  epilogue: output projection accumulated over all 4 pairs in PSUM
    (4 matmuls per [128,512] chunk), bf16 store; the first chunks
    overlap the last pair's attention.

Attention per (t, qj) follows the baseline: two heads packed into
disjoint 64-row PE groups for the ST matmuls (concurrent), one wide
ACTIVATE exp per k-tile amortizing the 352-cycle fixed cost, causal
mask as a triangular multiply on the diagonal subblock, PV with a
ones column so softmax sums ride along in row 64 of the PSUM
accumulator. Sums are normalized via DVE reciprocal on the [1,512]
row + a GpSimd partition-broadcast (the PE outer-product broadcast
and its PSUM bank are gone).
"""

import numpy as np

import concourse.bass as bass
import concourse.mybir as mybir
from concourse import bacc
from concourse.tile import TileContext

FP = mybir.dt.float32
BF = mybir.dt.bfloat16
P = 128


def build(S=2048, E=1024, HPC=8, DH=64, NQ=512):
    GD = HPC * DH  # 512 head dims per core
    KT_E = E // P  # 8 e-tiles
    ST_S = S // P  # 16 s-tiles
    QC = S // NQ  # 4 q-chunks
    DT = GD // P  # 4 head pairs
    QSUB = NQ // P  # 4 k-tiles per q-chunk step
    VW = 65
    SH = S // 2  # xT s-chunk width
    NCH = S // SH  # 2 s-chunks

    assert NQ % P == 0 and S % NQ == 0 and E % P == 0 and GD % P == 0
    assert DH == 64, "head slicing assumes DH=64"

    nc = bacc.Bacc(None, target_bir_lowering=False)
    x_d = nc.dram_tensor("x", [S, E], BF, kind="ExternalInput")
    wq_d = nc.dram_tensor("wq", [E, GD], BF, kind="ExternalInput")
    wk_d = nc.dram_tensor("wk", [E, GD], BF, kind="ExternalInput")
    wv_d = nc.dram_tensor("wv", [E, GD], BF, kind="ExternalInput")
    wo_d = nc.dram_tensor("wo", [GD, E], BF, kind="ExternalInput")
    out_d = nc.dram_tensor("out", [S, E], BF, kind="ExternalOutput")

    with TileContext(nc) as tc:
        with (
            tc.tile_pool(name="consts", bufs=1) as consts,
            tc.tile_pool(name="qkv", bufs=1) as qkv,
            tc.tile_pool(name="xT", bufs=1) as xT_pool,
            tc.tile_pool(name="wbuf", bufs=1) as wbuf,
            tc.tile_pool(name="attT", bufs=1) as attT_pool,
        ):
            ones_bf = consts.tile([P, 8], BF)
            nc.vector.memset(ones_bf[:], 1.0)
            # upper-triangular-inclusive multiplicative mask (valid k <= q)
            ut_mask = consts.tile([P, P], BF)
            nc.gpsimd.memset(ut_mask[:], 0.0)
            nc.gpsimd.affine_select(
                out=ut_mask[:], in_=ut_mask[:],
                compare_op=mybir.AluOpType.is_gt, fill=1.0,
                base=0, pattern=[[-1, P]], channel_multiplier=1,
            )

            qT = [qkv.tile([P, S], BF, tag=f"qT{t}", name=f"qT{t}") for t in range(DT)]
            kT = [qkv.tile([P, S], BF, tag=f"kT{t}", name=f"kT{t}") for t in range(DT)]
            v = [qkv.tile([P, HPC * VW], BF, tag=f"v{t}", name=f"v{t}") for t in range(ST_S)]
            attT = [
                attT_pool.tile([P, S], BF, tag=f"attT{t}", name=f"attT{t}")
                for t in range(DT)
            ]

            xTc = [
                [
                    xT_pool.tile([P, SH], BF, tag=f"xT{t}_{c}", name=f"xT{t}_{c}")
                    for c in range(NCH)
                ]
                for t in range(KT_E)
            ]

            def xT(kt, lo, width):
                c, r = divmod(lo, SH)
                assert r + width <= SH
                return xTc[kt][c][:, r : r + width]

            # ---- input DMA: everything on the sync HWDGE ring in need
            # order (the XBAR transpose unit is shared hardware and mixing
            # rings/modes either races or gets serialized behind the whole
            # transpose stream — one explicitly-ordered ring is both safe
            # and schedulable) ------------------------------------------
            hwdge = [nc.sync, nc.scalar]

            def w3_load(name, ap3):
                wt = wbuf.tile(
                    [P, ap3.shape[1] * ap3.shape[2]], BF, tag=name, name=name
                )
                w3 = wt.rearrange("p (k c) -> p k c", c=ap3.shape[2])
                nc.sync.dma_start(out=w3[:, :, :], in_=ap3)
                return w3

            # single ring, need-order: Tile's 8 DMA-completion sem lanes
            # are handed out round-robin in order, and each DMA waits on
            # its lane's previous user — so cross-ring interleaves
            # serialize anyway, and one explicitly-ordered ring wins
            # wv/wqk0 issue first so their transfer + completion latency
            # hides under the serial transpose stream (the first compute
            # chunk needs wv AND all 8 sc0 e-tiles; transposes dominate)
            wv3 = w3_load("wv", wv_d.rearrange("(k p) c -> p k c", p=P))
            wqk = {}
            for pname, w_d in (("k", wk_d), ("q", wq_d)):
                wqk[pname, 0] = w3_load(
                    f"w{pname}0",
                    w_d[:, 0:P].rearrange("(k p) c -> p k c", p=P),
                )
            for et in range(KT_E):
                nc.sync.dma_start(
                    out=xTc[et][0][:],
                    in_=x_d[0:SH, et * P : (et + 1) * P],
                    transpose=True,
                )
            for t in range(1, DT):
                for pname, w_d in (("k", wk_d), ("q", wq_d)):
                    wqk[pname, t] = w3_load(
                        f"w{pname}{t}",
                        w_d[:, t * P : (t + 1) * P].rearrange(
                            "(k p) c -> p k c", p=P
                        ),
                    )
            wo_sb = [
                wbuf.tile([P, E], BF, tag=f"wo{t}", name=f"wo{t}")
                for t in range(DT)
            ]
            for t in range(DT):
                nc.sync.dma_start(
                    out=wo_sb[t][:], in_=wo_d[t * P : (t + 1) * P, :]
                )
            for et in range(KT_E):
                nc.sync.dma_start(
                    out=xTc[et][1][:],
                    in_=x_d[SH : 2 * SH, et * P : (et + 1) * P],
                    transpose=True,
                )

            # ---- projection chunk emitters --------------------------------
            def v_chunk(pool, st, copy_eng):
                ps = pool.tile([P, GD], FP, tag="pj")
                for kt in range(KT_E):
                    nc.tensor.matmul(
                        ps[:],
                        lhsT=xT(kt, st * P, P),
                        rhs=wv3[:, kt, :],
                        start=(kt == 0),
                        stop=(kt == KT_E - 1),
                    )
                vst3 = v[st].rearrange("p (h c) -> p h c", c=VW)
                nc.vector.tensor_copy(
                    vst3[:, 0:HPC, 64:65], ones_bf[:, 0:HPC, None]
                )
                copy_eng(
                    vst3[:, :, 0:64], ps[:].rearrange("p (h c) -> p h c", c=DH)
                )

            def qk_chunk(pool, pname, t, nsc, copy_eng):
                dstT = kT if pname == "k" else qT
                ps = pool.tile([P, NQ], FP, tag="pj")
                w3 = wqk[pname, t]
                for kt in range(KT_E):
                    nc.tensor.matmul(
                        ps[:],
                        lhsT=w3[:, kt, :],
                        rhs=xT(kt, nsc * NQ, NQ),
                        start=(kt == 0),
                        stop=(kt == KT_E - 1),
                    )
                copy_eng(dstT[t][:, nsc * NQ : (nsc + 1) * NQ], ps[:])

            def scalar_copy(dst, src):
                nc.scalar.copy(dst, src)

            def vector_copy(dst, src):
                nc.vector.tensor_copy(dst, src)

            # ---- prologue: just enough projections to start attention ----
            # qk copies ride the idle ACT engine; the 3D-strided v copies
            # stay on the DVE
            with tc.tile_pool(name="pj_psum", bufs=4, space="PSUM") as pj_psum:
                qk_chunk(pj_psum, "k", 0, 0, scalar_copy)
                qk_chunk(pj_psum, "q", 0, 0, scalar_copy)
                for st in range(4):
                    v_chunk(pj_psum, st, vector_copy)

            # filler schedule: remaining projection chunks, emitted into
            # the attention qj loops as PE gap-fillers.  att(0) finishes
            # v and pair-0's q/k tails + pair-1's q/k; att(1) -> pair 2;
            # att(2) -> pair 3; att(3) overlaps the output projection.
            fillers = {
                (0, 0): [("v", 4), ("v", 5), ("v", 6), ("v", 7),
                         ("k", 0, 1), ("q", 0, 1)],
                (0, 1): [("v", 8), ("v", 9), ("v", 10), ("v", 11),
                         ("k", 0, 2), ("q", 0, 2)],
                (0, 2): [("v", 12), ("v", 13), ("v", 14), ("v", 15),
                         ("k", 0, 3), ("q", 0, 3)],
                (0, 3): [("k", 1, 0), ("q", 1, 0), ("k", 1, 1), ("q", 1, 1)],
                (1, 0): [("k", 1, 2), ("q", 1, 2), ("k", 1, 3), ("q", 1, 3)],
                (1, 1): [("k", 2, 0), ("q", 2, 0)],
                (1, 2): [("k", 2, 1), ("q", 2, 1), ("k", 2, 2), ("q", 2, 2)],
                (1, 3): [("k", 2, 3), ("q", 2, 3)],
                (2, 0): [("k", 3, 0), ("q", 3, 0)],
                (2, 1): [("k", 3, 1), ("q", 3, 1)],
                (2, 2): [("k", 3, 2), ("q", 3, 2)],
                (2, 3): [("k", 3, 3), ("q", 3, 3)],
            }

            # ---- attention + epilogue (PSUM: st 4 banks, av 2, sc 2) ----
            with (
                tc.tile_pool(name="st_psum", bufs=2, space="PSUM") as st_psum,
                tc.tile_pool(name="av_psum", bufs=2, space="PSUM") as av_psum,
                tc.tile_pool(name="sc_psum", bufs=2, space="PSUM") as sc_psum,
                tc.tile_pool(name="pt_sb", bufs=10) as pt_pool,
                tc.tile_pool(name="nrm_sb", bufs=4) as nrm_pool,
                tc.tile_pool(name="ostage", bufs=4) as ostage,
            ):
                di = [0]

                def outproj_chunk(st, nj):
                    po = sc_psum.tile([P, NQ], FP, tag="pj")
                    for tt in range(DT):
                        nc.tensor.matmul(
                            po[:],
                            lhsT=attT[tt][:, st * P : (st + 1) * P],
                            rhs=wo_sb[tt][:, nj * NQ : (nj + 1) * NQ],
                            start=(tt == 0), stop=(tt == DT - 1),
                        )
                    og = ostage.tile([P, NQ], BF, tag="og")
                    nc.vector.tensor_copy(og[:], po[:])
                    hwdge[di[0] % 2].dma_start(
                        out=out_d[st * P : (st + 1) * P,
                                  nj * NQ : (nj + 1) * NQ],
                        in_=og[:],
                    )
                    di[0] += 1

                # outproj chunks st-major; st-block st is final once
                # att(3) qj = st//4 normalized, so inside att(3) chunks up
                # to st < 4*qj are safe to emit (one-qj lag)
                op_iter = [(st, nj) for st in range(ST_S)
                           for nj in range(E // NQ)]
                op_pos = [0]

                def emit_op_chunks(st_limit, n):
                    while (
                        n > 0 and op_pos[0] < len(op_iter)
                        and op_iter[op_pos[0]][0] < st_limit
                    ):
                        st, nj = op_iter[op_pos[0]]
                        op_pos[0] += 1
                        outproj_chunk(st, nj)
                        n -= 1

                def emit_fillers(t, qj):
                    for f in fillers.get((t, qj), ()):
                        if f[0] == "v":
                            v_chunk(sc_psum, f[1], vector_copy)
                        else:
                            qk_chunk(sc_psum, f[0], f[1], f[2], vector_copy)


                for t in range(DT):
                    # head pair (2t, 2t+1): partition rows 0-63 / 64-127 of
                    # kT[t]/qT[t].  The two ST matmuls use disjoint 64-row
                    # groups of the PE array and run concurrently.
                    attT_t = attT[t]
                    for qj in range(QC):
                        avs = [
                            av_psum.tile([VW, NQ], FP, tag="av",
                                         name=f"av{t}_{qj}_{half}")
                            for half in range(2)
                        ]
                        n_tiles = QSUB * qj + QSUB
                        kmax = n_tiles - 1
                        for ki in range(n_tiles):
                            d = ki - QSUB * qj
                            off = P * d if d > 0 else 0
                            stp = st_psum.tile([P, 2 * NQ], FP, tag="st")
                            for half in range(2):
                                pr = 64 * half
                                nc.tensor.matmul(
                                    stp[:, half * NQ + off : (half + 1) * NQ],
                                    lhsT=kT[t][pr : pr + 64, ki * P : (ki + 1) * P],
                                    rhs=qT[t][pr : pr + 64, qj * NQ + off : (qj + 1) * NQ],
                                    start=True, stop=True,
                                )
                            pt = pt_pool.tile([P, 2 * NQ], BF, tag="pt")
                            if off == 0:
                                nc.scalar.activation(
                                    pt[:, 0 : 2 * NQ], stp[:, 0 : 2 * NQ],
                                    mybir.ActivationFunctionType.Exp,
                                    scale=0.125,
                                )
                            else:
                                # one ACTIVATE over both heads' valid spans
                                # via a strided AP; dead cols are never read
                                # (the PV matmuls below are col-restricted)
                                pt2 = pt.rearrange("p (k c) -> p k c", c=NQ)
                                st2 = stp.rearrange("p (k c) -> p k c", c=NQ)
                                nc.scalar.activation(
                                    pt2[:, :, off:NQ], st2[:, :, off:NQ],
                                    mybir.ActivationFunctionType.Exp,
                                    scale=0.125,
                                )
                            if d >= 0:
                                for half in range(2):
                                    nc.vector.tensor_tensor(
                                        pt[:, half * NQ + off : half * NQ + off + P],
                                        pt[:, half * NQ + off : half * NQ + off + P],
                                        ut_mask[:],
                                        mybir.AluOpType.mult,
                                    )
                            for half in range(2):
                                h = 2 * t + half
                                # col-restricted: cols < off are fully masked
                                # and were already initialized by ki=0
                                nc.tensor.matmul(
                                    avs[half][:, off:NQ],
                                    lhsT=v[ki][:, VW * h : VW * h + VW],
                                    rhs=pt[:, half * NQ + off : (half + 1) * NQ],
                                    start=(ki == 0), stop=(ki == kmax),
                                    skip_group_check=True,
                                )
                            if t == DT - 1 and ki % 2 == 1:
                                # drip output-projection chunks into att(3)'s
                                # ACT-bound PE slack (no qk fillers left here;
                                # without them the HAM gate re-throttles)
                                emit_op_chunks(4 * qj, 1)
                        for half in range(2):
                            pr = 64 * half
                            av = avs[half]
                            # softmax sums ride in row 64; move them to
                            # partition 0 (plain copy handles the cross-
                            # partition hop; custom DVE ops are lane-wired),
                            # reciprocal there, broadcast on the idle GpSimd
                            sums = nrm_pool.tile([1, NQ], FP, tag="sums")
                            nc.vector.tensor_copy(sums[:], av[64:65, :])
                            sums_r = nrm_pool.tile([1, NQ], FP, tag="sumr")
                            nc.vector.reciprocal_approx_fast(
                                sums_r[:], sums[:]
                            )
                            bc = nrm_pool.tile([64, NQ], FP, tag="bc")
                            nc.gpsimd.partition_broadcast(
                                bc[:], sums_r[:], channels=64
                            )
                            nc.vector.tensor_tensor(
                                attT_t[pr : pr + 64, qj * NQ : (qj + 1) * NQ],
                                av[0:64, :],
                                bc[:],
                                mybir.AluOpType.mult,
                            )
                        emit_fillers(t, qj)

                # remaining output-projection chunks drain at the end
                emit_op_chunks(ST_S, len(op_iter))


    nc.compile()
    return nc


_NC_CACHE = {}


def _get_nc():
    if "nc" not in _NC_CACHE:
        _NC_CACHE["nc"] = build()
    return _NC_CACHE["nc"]


B, S, E, H, DH = 4, 2048, 1024, 16, 64
GD = (H // 2) * DH  # 512 per-core head dims


def _in_maps(x, Wq, Wk, Wv, Wo):
    import ml_dtypes

    bf = ml_dtypes.bfloat16
    maps = []
    for c in range(8):
        b, g = c // 2, c % 2
        sl = slice(g * GD, (g + 1) * GD)
        maps.append({
            "x": x[b].astype(bf),
            "wq": Wq[:, sl].astype(bf),
            "wk": Wk[:, sl].astype(bf),
            "wv": Wv[:, sl].astype(bf),
            "wo": Wo[sl, :].astype(bf),
        })
    return maps


def kernel(x, Wq, Wk, Wv, Wo):
    from concourse.bass_utils import run_bass_kernel_spmd

    x = np.asarray(x, dtype=np.float32)
    Wq = np.asarray(Wq, dtype=np.float32)
    Wk = np.asarray(Wk, dtype=np.float32)
    Wv = np.asarray(Wv, dtype=np.float32)
    Wo = np.asarray(Wo, dtype=np.float32)

    res = run_bass_kernel_spmd(
        _get_nc(), _in_maps(x, Wq, Wk, Wv, Wo), list(range(8))
    )

    out = np.empty((B, S, E), np.float32)
    for b in range(B):
        out[b] = res.results[2 * b]["out"].astype(np.float32) + res.results[
            2 * b + 1
        ]["out"].astype(np.float32)
    return out
